# revision 1
# baseline (speedup 1.0000x reference)
"""LSTM encoder (embedding gather + 512-step LSTM) on 8 TRN2 NeuronCores.

Sharding: data-parallel over batch — each of the 8 cores owns 8 of the 64
sequences end-to-end (embedding table and weights replicated), so the
sequential recurrence needs no cross-core communication.

Per-core Bass/Tile kernel:
  Phase 1 (x-phase): indirect-DMA gather of embedding rows, PE-transpose to
    X.T, GEMM xg = X @ W_ih + b -> xg_hbm [S, 32, H] bf16 (dim1 = (gate
    strip j, batch b), strips ordered i, f, o, g).
  Phase 2 (recurrence): hardware For_i loop over S/U iterations, U steps
    unrolled per iteration. Per step:
      - scatter-matmul seeds the gate PSUM [128, 1024] with xg (rows 32j+b)
      - 64 col-strip-packed matmuls accumulate h.T @ W_hh (tile_position)
      - evacuate -> bf16, 8 PE transposes -> hidden-partition gate layout
      - ACT sigmoid/tanh + DVE cell update produce c, h; h.T feeds the next
        step's stationary operand directly.
All matmuls are bf16 with fp32 PSUM accumulation; the cell state is fp32.
"""
import sys

if "/opt/trn_rl_repo" not in sys.path:
    sys.path.insert(0, "/opt/trn_rl_repo")

import numpy as np
import ml_dtypes
import concourse.bass as bass
import concourse.tile as tile
from concourse import bacc, mybir
from concourse.masks import make_identity

F32 = mybir.dt.float32
BF16 = mybir.dt.bfloat16
I32 = mybir.dt.int32
P = 128
GATE_PERM = [0, 1, 3, 2]  # strip j -> original gate block (W order: i, f, g, o)

# Problem constants (hardcoded per contest contract)
VOCAB, E, H = 32000, 1024, 1024
B, S = 64, 512
NCORES = 8
BLOC = B // NCORES
U = 16

_program_cache = {}


def build_program(S=S, BLOC=BLOC, E=E, H=H, VOCAB=VOCAB, U=U):
    """x-phase interleaved into the recurrence: iteration iv computes the
    x-gates m-tile for iteration iv+1 in the PE gaps between steps.
    Requires U == 16 (one 128-token m-tile per iteration)."""
    KT = E // P
    KTH = H // P
    GN = 4 * H
    TOK = S * BLOC
    NIT = S // U
    JB = 4 * BLOC
    assert U == 16 and S % U == 0 and TOK // P == NIT

    nc = bacc.Bacc(None, target_bir_lowering=False, debug=False)

    src_idx = nc.dram_tensor("src_idx", [TOK + P, 1], I32, kind="ExternalInput")
    emb = nc.dram_tensor("emb", [VOCAB, E], F32, kind="ExternalInput")
    wih = nc.dram_tensor("wih", [P, KT, GN], BF16, kind="ExternalInput")
    whh = nc.dram_tensor("whh", [P, KTH, GN], BF16, kind="ExternalInput")
    bias = nc.dram_tensor("bias", [GN], F32, kind="ExternalInput")
    scat = nc.dram_tensor("scat", [JB, P], BF16, kind="ExternalInput")
    hs = nc.dram_tensor("hs", [S, P, BLOC * KTH], BF16, kind="ExternalOutput")
    xg_hbm = nc.dram_tensor("xg_hbm", [S + U, JB, H], BF16)

    with tile.TileContext(nc) as tc:
        with tc.tile_pool(name="const", bufs=1) as const, \
             tc.tile_pool(name="rw", bufs=1) as rw, \
             tc.tile_pool(name="state", bufs=1) as state, \
             tc.tile_pool(name="rsb", bufs=2) as rsb, \
             tc.tile_pool(name="rps", bufs=2, space="PSUM") as rps, \
             tc.tile_pool(name="gtps_pool", bufs=1, space="PSUM") as gtps_pool, \
             tc.tile_pool(name="xtp", bufs=2, space="PSUM") as xtp, \
             tc.tile_pool(name="xgp", bufs=1, space="PSUM") as xgp:
            ident = const.tile([P, P], BF16)
            make_identity(nc, ident[:])
            whh_sb = rw.tile([P, KTH, GN], BF16)
            nc.sync.dma_start(out=whh_sb[:], in_=whh[:])
            wih_sb = rw.tile([P, KT, GN], BF16)
            nc.sync.dma_start(out=wih_sb[:], in_=wih[:])
            bias_sb = rw.tile([P, GN], F32)
            nc.sync.dma_start(out=bias_sb[:], in_=bass.AP(
                tensor=bias.ap().tensor, offset=0, ap=[[0, P], [1, GN]]))
            scat_sb = rw.tile([JB, P], BF16)
            nc.sync.dma_start(out=scat_sb[:], in_=scat[:])

            hT = [state.tile([P, KTH * 32], BF16, tag=f"hT{i}", name=f"hT{i}")
                  for i in range(2)]
            cst = [state.tile([P, BLOC * KTH], F32, tag=f"cst{i}", name=f"cst{i}")
                   for i in range(2)]
            nc.vector.memset(hT[0][:], 0.0)
            nc.vector.memset(hT[1][:], 0.0)
            nc.vector.memset(cst[0][:], 0.0)
            xg_it = state.tile([JB, U * H], BF16, tag="xgit")
            hs_it = state.tile([P, U * BLOC * KTH], BF16, tag="hsit")
            idx_sb = state.tile([P, 1], I32, tag="idx")
            xrow = state.tile([P, E], F32, tag="xrow")
            xrow_bf = state.tile([P, E], BF16, tag="xrowbf")
            xt_sb = state.tile([P, KT * P], BF16, tag="xt")

            def x_chunk(u, mt):
                """Emit slice u (0..15) of the x-phase work for m-tile mt
                (int or ScalarValue)."""
                if u == 0:
                    nc.sync.dma_start(out=idx_sb[:],
                                      in_=src_idx[bass.ds(mt * P, P), :])
                    nc.gpsimd.indirect_dma_start(
                        out=xrow[:], out_offset=None, in_=emb[:],
                        in_offset=bass.IndirectOffsetOnAxis(ap=idx_sb[:, :1], axis=0))
                elif u == 1:
                    nc.vector.tensor_copy(out=xrow_bf[:], in_=xrow[:])
                elif 2 <= u <= 5:
                    for c in (2 * (u - 2), 2 * (u - 2) + 1):
                        xt_ps = xtp.tile([P, P], BF16, tag="xtps")
                        nc.tensor.transpose(out=xt_ps[:],
                                            in_=xrow_bf[:, c * P:(c + 1) * P],
                                            identity=ident[:])
                        nc.scalar.copy(out=xt_sb[:, c * P:(c + 1) * P], in_=xt_ps[:])
                elif 6 <= u <= 13:
                    jn = u - 6
                    j, nh = jn // 2, jn % 2
                    xg_ps = xgp.tile([P, 512], F32, tag="xgps")
                    for k in range(KT):
                        nc.tensor.matmul(
                            out=xg_ps[:], lhsT=xt_sb[:, k * P:(k + 1) * P],
                            rhs=wih_sb[:, k, jn * 512:(jn + 1) * 512],
                            start=(k == 0), stop=(k == KT - 1))
                    xgq = rsb.tile([P, 512], BF16, tag="xgq")
                    nc.vector.tensor_tensor(out=xgq[:], in0=xg_ps[:],
                                            in1=bias_sb[:, jn * 512:(jn + 1) * 512],
                                            op=mybir.AluOpType.add)
                    nc.sync.dma_start(
                        out=xg_hbm[bass.ds(mt * (P // BLOC), P // BLOC),
                                   j * BLOC:(j + 1) * BLOC,
                                   nh * 512:(nh + 1) * 512],
                        in_=xgq[:])

            def step(u):
                h_cur, h_new = hT[u % 2], hT[(u + 1) % 2]
                c_cur, c_new = cst[u % 2], cst[(u + 1) % 2]
                g_ps = rps.tile([P, 1024], F32, tag="gps")
                g_sb = rsb.tile([P, 1024], BF16, tag="gsb")
                for n in range(2):
                    nc.tensor.matmul(
                        out=g_ps[:, 512 * n:512 * (n + 1)],
                        lhsT=scat_sb[:, :],
                        rhs=xg_it[:, u * H + 512 * n: u * H + 512 * (n + 1)],
                        start=True, stop=True)
                # k-outer with n interleaved: consecutive MMs alternate PSUM
                # banks (and col strips), which paces the PE drain pipeline
                # measurably better than finishing one bank first.
                for k in range(KTH):
                    for n in range(2):
                        for j in range(4):
                            nc.tensor.matmul(
                                out=g_ps[32 * j:32 * (j + 1), 512 * n:512 * (n + 1)],
                                lhsT=h_cur[:, 32 * k:32 * (k + 1)],
                                rhs=whh_sb[:, k, j * H + 512 * n: j * H + 512 * (n + 1)],
                                start=False, stop=(k == KTH - 1),
                                tile_position=(0, 32 * j),
                                skip_group_check=True)
                nc.scalar.copy(out=g_sb[:, 0:512], in_=g_ps[:, 0:512])
                gt_ps = gtps_pool.tile([P, 1024], BF16, tag="gtps")
                for c in range(KTH // 2):
                    nc.tensor.transpose(out=gt_ps[:, c * P:(c + 1) * P],
                                        in_=g_sb[:, c * P:(c + 1) * P], identity=ident[:])
                nc.vector.tensor_copy(out=g_sb[:, 512:768], in_=g_ps[:, 512:768])
                nc.scalar.copy(out=g_sb[:, 768:1024], in_=g_ps[:, 768:1024])
                for c in range(KTH // 2, KTH):
                    nc.tensor.transpose(out=gt_ps[:, c * P:(c + 1) * P],
                                        in_=g_sb[:, c * P:(c + 1) * P], identity=ident[:])
                base = gt_ps[:]

                def gt_src(j0, nj):
                    return bass.AP(tensor=base.tensor, offset=base.offset + 32 * j0,
                                   ap=[base.ap[0], [32, nj], [P, KTH], [1, BLOC]])

                s_ifo = rsb.tile([P, 192], F32, tag="sifo")
                nc.scalar.activation(out=s_ifo[:].rearrange("p (j c b) -> p j c b", j=3, c=KTH),
                                     in_=gt_src(0, 3),
                                     func=mybir.ActivationFunctionType.Sigmoid)
                t_g = rsb.tile([P, 64], F32, tag="tg")
                nc.scalar.activation(out=t_g[:].rearrange("p (j c b) -> p j c b", j=1, c=KTH),
                                     in_=gt_src(3, 1),
                                     func=mybir.ActivationFunctionType.Tanh)
                fc = rsb.tile([P, 64], F32, tag="fc")
                nc.vector.tensor_tensor(out=fc[:], in0=c_cur[:], in1=s_ifo[:, 64:128],
                                        op=mybir.AluOpType.mult)
                ig = rsb.tile([P, 64], F32, tag="ig")
                nc.vector.tensor_tensor(out=ig[:], in0=t_g[:], in1=s_ifo[:, 0:64],
                                        op=mybir.AluOpType.mult)
                nc.vector.tensor_tensor(out=c_new[:], in0=fc[:], in1=ig[:],
                                        op=mybir.AluOpType.add)
                t_c = rsb.tile([P, 64], F32, tag="tc")
                nc.scalar.activation(out=t_c[:], in_=c_new[:],
                                     func=mybir.ActivationFunctionType.Tanh)
                hout = hs_it[:, u * 64:(u + 1) * 64]
                nc.vector.tensor_tensor(out=hout, in0=t_c[:], in1=s_ifo[:, 128:192],
                                        op=mybir.AluOpType.mult)
                hT_dst = bass.AP(tensor=h_new.tensor, offset=h_new[:].offset,
                                 ap=[h_new[:].ap[0], [32, KTH], [1, BLOC]])
                nc.vector.tensor_copy(
                    out=hT_dst,
                    in_=hs_it[:, u * 64:(u + 1) * 64].rearrange("p (c b) -> p c b", c=KTH))

            # prologue: x m-tile 0
            for u in range(U):
                x_chunk(u, 0)

            with tc.For_i(0, NIT, 1) as iv:
                nc.sync.dma_start(
                    out=xg_it[:].rearrange("p (t h) -> p t h", t=U),
                    in_=xg_hbm[bass.ds(iv * U, U), :, :].rearrange("t p h -> p t h"))
                for u in range(U):
                    step(u)
                    x_chunk(u, iv + 1)
                nc.sync.dma_start(
                    out=hs[bass.ds(iv * U, U), :, :].rearrange("t p c -> p t c"),
                    in_=hs_it[:].rearrange("p (t c) -> p t c", t=U))

    nc.compile()
    return nc


def _prep_inputs(source, embedding, W_ih, W_hh, b, core, n_cores=NCORES):
    src_k = np.asarray(source[core * BLOC:(core + 1) * BLOC, :], dtype=np.int32)
    idx = np.ascontiguousarray(src_k.T.reshape(-1, 1))  # (t-major, b)
    idx = np.concatenate([idx, np.zeros((P, 1), np.int32)], axis=0)  # slack m-tile

    def prep_w(W, K):
        Wr = np.asarray(W, np.float32).reshape(K // P, P, 4, H)[:, :, GATE_PERM, :]
        return np.ascontiguousarray(
            Wr.transpose(1, 0, 2, 3).reshape(P, K // P, 4 * H)).astype(ml_dtypes.bfloat16)

    bias_dev = np.ascontiguousarray(
        np.asarray(b, np.float32).reshape(4, H)[GATE_PERM].reshape(4 * H))
    JB = 4 * BLOC
    scat = np.zeros((JB, P), np.float32)
    for j in range(4):
        for bb in range(BLOC):
            scat[j * BLOC + bb, 32 * j + bb] = 1.0
    return {
        "src_idx": idx,
        "emb": np.asarray(embedding, np.float32),
        "wih": prep_w(W_ih, E),
        "whh": prep_w(W_hh, H),
        "bias": bias_dev,
        "scat": scat.astype(ml_dtypes.bfloat16),
    }


def _unpack_output(hs_dev):
    KTH = H // P
    a = np.asarray(hs_dev, dtype=np.float32).reshape(S, P, KTH, BLOC)
    return np.ascontiguousarray(a.transpose(3, 0, 2, 1)).reshape(BLOC, S, H)


# Weight prep is deterministic; cache per-core input maps keyed on id of arrays.
def _get_program():
    if "nc" not in _program_cache:
        _program_cache["nc"] = build_program()
    return _program_cache["nc"]


def kernel(source, embedding, W_ih, W_hh, b):
    """Full inputs in, full output out. Shards batch over 8 NeuronCores."""
    from concourse import bass2jax

    source = np.asarray(source)
    embedding = np.asarray(embedding, np.float32)
    W_ih = np.asarray(W_ih, np.float32)
    W_hh = np.asarray(W_hh, np.float32)
    b = np.asarray(b, np.float32)

    nc = _get_program()
    in_maps = [_prep_inputs(source, embedding, W_ih, W_hh, b, core=k)
               for k in range(NCORES)]
    res = bass2jax.run_bass_via_pjrt(nc, in_maps, n_cores=NCORES)
    out = np.concatenate([_unpack_output(res[k]["hs"]) for k in range(NCORES)],
                         axis=0)
    return out.astype(np.float32)



# revision 15
# speedup vs baseline: 1.2649x; 1.2649x over previous
"""LSTM encoder (embedding gather + 512-step LSTM) on 8 TRN2 NeuronCores.

Sharding: data-parallel over batch — each of the 8 cores owns 8 of the 64
sequences end-to-end (embedding table and weights replicated), so the
sequential recurrence needs no cross-core communication.

Per-core Bass/Tile kernel:
  Phase 1 (x-phase): indirect-DMA gather of embedding rows, PE-transpose to
    X.T, GEMM xg = X @ W_ih + b -> xg_hbm [S, 32, H] bf16 (dim1 = (gate
    strip j, batch b), strips ordered i, f, o, g).
  Phase 2 (recurrence): hardware For_i loop over S/U iterations, U steps
    unrolled per iteration. Per step:
      - scatter-matmul seeds the gate PSUM [128, 1024] with xg (rows 32j+b)
      - 64 col-strip-packed matmuls accumulate h.T @ W_hh (tile_position)
      - evacuate -> bf16, 8 PE transposes -> hidden-partition gate layout
      - ACT sigmoid/tanh + DVE cell update produce c, h; h.T feeds the next
        step's stationary operand directly.
All matmuls are bf16 with fp32 PSUM accumulation; the cell state is fp32.
"""
import sys

if "/opt/trn_rl_repo" not in sys.path:
    sys.path.insert(0, "/opt/trn_rl_repo")

import numpy as np
import ml_dtypes
import concourse.bass as bass
import concourse.tile as tile
from concourse import bacc, mybir
from concourse.masks import make_identity

F32 = mybir.dt.float32
BF16 = mybir.dt.bfloat16
I32 = mybir.dt.int32
P = 128
GATE_PERM = [0, 1, 3, 2]  # strip j -> original gate block (W order: i, f, g, o)

# Problem constants (hardcoded per contest contract)
VOCAB, E, H = 32000, 1024, 1024
B, S = 64, 512
NCORES = 8
BLOC = B // NCORES
U = 16

_program_cache = {}


def build_program(S=S, BLOC=BLOC, E=E, H=H, VOCAB=VOCAB, U=U):
    """x-phase interleaved into the recurrence: iteration iv computes the
    x-gates m-tile for iteration iv+1 in the PE gaps between steps.
    Requires U == 16 (one 128-token m-tile per iteration)."""
    KT = E // P
    KTH = H // P
    GN = 4 * H
    TOK = S * BLOC
    NIT = S // U
    JB = 4 * BLOC
    assert U == 16 and S % U == 0 and TOK // P == NIT

    nc = bacc.Bacc(None, target_bir_lowering=False, debug=False)

    src_idx = nc.dram_tensor("src_idx", [TOK + P, 1], I32, kind="ExternalInput")
    emb = nc.dram_tensor("emb", [VOCAB, E], F32, kind="ExternalInput")
    wih = nc.dram_tensor("wih", [P, KT, GN], BF16, kind="ExternalInput")
    whh = nc.dram_tensor("whh", [P, KTH, GN], BF16, kind="ExternalInput")
    bias_rows = nc.dram_tensor("bias_rows", [4, U * H], BF16, kind="ExternalInput")
    scat = nc.dram_tensor("scat", [JB + 4, P], BF16, kind="ExternalInput")
    hs = nc.dram_tensor("hs", [S, P, BLOC * KTH], BF16, kind="ExternalOutput")
    xg_hbm = nc.dram_tensor("xg_hbm", [S + U, JB, H], BF16)

    with tile.TileContext(nc) as tc:
        with tc.tile_pool(name="const", bufs=1) as const, \
             tc.tile_pool(name="rw", bufs=1) as rw, \
             tc.tile_pool(name="state", bufs=1) as state, \
             tc.tile_pool(name="rsb", bufs=2) as rsb, \
             tc.tile_pool(name="rps", bufs=2, space="PSUM") as rps, \
             tc.tile_pool(name="gtps_pool", bufs=1, space="PSUM") as gtps_pool, \
             tc.tile_pool(name="xtp", bufs=2, space="PSUM") as xtp, \
             tc.tile_pool(name="xgp", bufs=1, space="PSUM") as xgp:
            ident = const.tile([P, P], BF16)
            make_identity(nc, ident[:])
            whh_sb = rw.tile([P, KTH, GN], BF16)
            nc.sync.dma_start(out=whh_sb[:], in_=whh[:])
            wih_sb = rw.tile([P, KT, GN], BF16)
            nc.sync.dma_start(out=wih_sb[:], in_=wih[:])
            scat_sb = rw.tile([JB + 4, P], BF16)
            nc.sync.dma_start(out=scat_sb[:], in_=scat[:])

            hT = [state.tile([P, KTH * 32], BF16, tag=f"hT{i}", name=f"hT{i}")
                  for i in range(2)]
            cst = [state.tile([P, BLOC * KTH], F32, tag=f"cst{i}", name=f"cst{i}")
                   for i in range(2)]
            nc.vector.memset(hT[0][:], 0.0)
            nc.vector.memset(hT[1][:], 0.0)
            nc.vector.memset(cst[0][:], 0.0)
            xg_it = state.tile([JB + 4, U * H], BF16, tag="xgit")
            # rows JB..JB+3: per-gate-strip bias, tiled per step; the seed
            # matmul's extra scatter rows add it into the gate PSUM for free.
            nc.sync.dma_start(out=xg_it[JB:JB + 4, :], in_=bias_rows[:])
            hs_it = state.tile([P, U * BLOC * KTH], BF16, tag="hsit")
            idx_sb = state.tile([P, 1], I32, tag="idx")
            xrow = state.tile([P, E], F32, tag="xrow")
            xrow_bf = state.tile([P, E], BF16, tag="xrowbf")
            xt_sb = state.tile([P, KT * P], BF16, tag="xt")

            def x_chunk(u, mt):
                """Emit slice u (0..15) of the x-phase work for m-tile mt
                (int or ScalarValue)."""
                if u == 0:
                    nc.sync.dma_start(out=idx_sb[:],
                                      in_=src_idx[bass.ds(mt * P, P), :])
                    nc.gpsimd.indirect_dma_start(
                        out=xrow[:], out_offset=None, in_=emb[:],
                        in_offset=bass.IndirectOffsetOnAxis(ap=idx_sb[:, :1], axis=0))
                elif u == 1:
                    nc.vector.tensor_copy(out=xrow_bf[:], in_=xrow[:])
                elif 2 <= u <= 5:
                    for c in (2 * (u - 2), 2 * (u - 2) + 1):
                        xt_ps = xtp.tile([P, P], BF16, tag="xtps")
                        nc.tensor.transpose(out=xt_ps[:],
                                            in_=xrow_bf[:, c * P:(c + 1) * P],
                                            identity=ident[:])
                        nc.scalar.copy(out=xt_sb[:, c * P:(c + 1) * P], in_=xt_ps[:])
                elif 6 <= u <= 13:
                    x_mm(u - 6, mt)

            xg_pend = []

            def x_mm(jn, mt):
                """xg matmuls for slice jn of m-tile mt; evacuation is deferred
                to a later step's filler so it never gates the cell chain."""
                xg_ps = xgp.tile([P, 512], F32, tag="xgps")
                for k in range(KT):
                    nc.tensor.matmul(
                        out=xg_ps[:], lhsT=xt_sb[:, k * P:(k + 1) * P],
                        rhs=wih_sb[:, k, jn * 512:(jn + 1) * 512],
                        start=(k == 0), stop=(k == KT - 1))
                xg_pend.append((xg_ps, mt, jn))

            def x_out():
                if not xg_pend:
                    return
                xg_ps, mt, jn = xg_pend.pop(0)
                j, nh = jn // 2, jn % 2
                xgq = rsb.tile([P, 512], BF16, tag="xgq")
                nc.vector.tensor_copy(out=xgq[:], in_=xg_ps[:])
                nc.sync.dma_start(
                    out=xg_hbm[bass.ds(mt * (P // BLOC), P // BLOC),
                               j * BLOC:(j + 1) * BLOC,
                               nh * 512:(nh + 1) * 512],
                    in_=xgq[:])

            pend = {}

            def seeds(u):
                """Allocate step u's gate PSUM and seed it with xg via the
                scatter matmul. Called one step early (from step u-1's body)
                so the seed MMs fill the PE gap after the transposes."""
                g_ps = rps.tile([P, 1024], F32, tag="gps")
                for n in range(2):
                    nc.tensor.matmul(
                        out=g_ps[:, 512 * n:512 * (n + 1)],
                        lhsT=scat_sb[:, :],
                        rhs=xg_it[:, u * H + 512 * n: u * H + 512 * (n + 1)],
                        start=True, stop=True)
                pend[u] = g_ps

            def step(u, filler=None, filler_has_mms=False):
                h_cur, h_new = hT[u % 2], hT[(u + 1) % 2]
                c_cur, c_new = cst[u % 2], cst[(u + 1) % 2]
                if u not in pend:
                    seeds(u)
                g_ps = pend.pop(u)
                g_sb = rsb.tile([P, 1024], BF16, tag="gsb")
                # k-outer with n interleaved: consecutive MMs alternate PSUM
                # banks (and col strips), which paces the PE drain pipeline
                # measurably better than finishing one bank first.
                for k in range(KTH):
                    for n in range(2):
                        for j in range(4):
                            nc.tensor.matmul(
                                out=g_ps[32 * j:32 * (j + 1), 512 * n:512 * (n + 1)],
                                lhsT=h_cur[:, 32 * k:32 * (k + 1)],
                                rhs=whh_sb[:, k, j * H + 512 * n: j * H + 512 * (n + 1)],
                                start=False, stop=(k == KTH - 1),
                                tile_position=(0, 32 * j),
                                skip_group_check=True)
                nc.scalar.copy(out=g_sb[:, 0:512], in_=g_ps[:, 0:512])
                gt_ps = gtps_pool.tile([P, 1024], BF16, tag="gtps")
                for c in range(KTH // 2):
                    nc.tensor.transpose(out=gt_ps[:, c * P:(c + 1) * P],
                                        in_=g_sb[:, c * P:(c + 1) * P], identity=ident[:])
                nc.vector.tensor_copy(out=g_sb[:, 512:768], in_=g_ps[:, 512:768])
                nc.scalar.copy(out=g_sb[:, 768:1024], in_=g_ps[:, 768:1024])
                for c in range(KTH // 2, KTH):
                    nc.tensor.transpose(out=gt_ps[:, c * P:(c + 1) * P],
                                        in_=g_sb[:, c * P:(c + 1) * P], identity=ident[:])
                # Seed next step's gate PSUM now: the seed MMs fill the PE gap
                # between this step's transposes and the chain-gated work below.
                if u + 1 < U:
                    seeds(u + 1)
                base = gt_ps[:]

                def gt_src(j0, nj):
                    return bass.AP(tensor=base.tensor, offset=base.offset + 32 * j0,
                                   ap=[base.ap[0], [32, nj], [P, KTH], [1, BLOC]])

                s_ifo = rsb.tile([P, 192], F32, tag="sifo")
                nc.scalar.activation(out=s_ifo[:].rearrange("p (j c b) -> p j c b", j=3, c=KTH),
                                     in_=gt_src(0, 3),
                                     func=mybir.ActivationFunctionType.Sigmoid)
                # HAM keep-warm: a tiny bf16 matmul gated on mid-chain data
                # keeps the PE from idling a full MID window during the ACT/DVE
                # chain (the idle re-throttles the clock to 1.2 GHz and makes
                # the next step's first ~3.4us of matmuls run at half rate).
                # bf16 (not fp32) so the FWL weight path of subsequent bf16
                # matmuls is unaffected. Output goes to this step's
                # already-evacuated gate PSUM (dead data).
                warm_bf = rsb.tile([P, 32], BF16, tag="warm")
                nc.vector.tensor_copy(out=warm_bf[:], in_=s_ifo[:, 0:32])
                nc.tensor.matmul(out=g_ps[0:32, 0:32], lhsT=warm_bf[:],
                                 rhs=warm_bf[:], start=True, stop=True)
                if filler is not None:
                    filler()
                t_g = rsb.tile([P, 64], F32, tag="tg")
                nc.scalar.activation(out=t_g[:].rearrange("p (j c b) -> p j c b", j=1, c=KTH),
                                     in_=gt_src(3, 1),
                                     func=mybir.ActivationFunctionType.Tanh)
                fc = rsb.tile([P, 64], F32, tag="fc")
                nc.vector.tensor_tensor(out=fc[:], in0=c_cur[:], in1=s_ifo[:, 64:128],
                                        op=mybir.AluOpType.mult)
                ig = rsb.tile([P, 64], F32, tag="ig")
                nc.vector.tensor_tensor(out=ig[:], in0=t_g[:], in1=s_ifo[:, 0:64],
                                        op=mybir.AluOpType.mult)
                nc.vector.tensor_tensor(out=c_new[:], in0=fc[:], in1=ig[:],
                                        op=mybir.AluOpType.add)
                t_c = rsb.tile([P, 64], F32, tag="tc")
                nc.scalar.activation(out=t_c[:], in_=c_new[:],
                                     func=mybir.ActivationFunctionType.Tanh)
                hout = hs_it[:, u * 64:(u + 1) * 64]
                nc.vector.tensor_tensor(out=hout, in0=t_c[:], in1=s_ifo[:, 128:192],
                                        op=mybir.AluOpType.mult)
                if not filler_has_mms:
                    # Second keep-warm MM late in the chain for steps whose
                    # x-phase filler has no matmul work.
                    nc.tensor.matmul(out=g_ps[0:32, 64:128],
                                     lhsT=hs_it[:, u * 64:u * 64 + 32],
                                     rhs=hs_it[:, u * 64:(u + 1) * 64],
                                     start=True, stop=True)
                hT_dst = bass.AP(tensor=h_new.tensor, offset=h_new[:].offset,
                                 ap=[h_new[:].ap[0], [32, KTH], [1, BLOC]])
                nc.vector.tensor_copy(
                    out=hT_dst,
                    in_=hs_it[:, u * 64:(u + 1) * 64].rearrange("p (c b) -> p c b", c=KTH))

            # prologue: x m-tile 0
            for u in range(U):
                x_chunk(u, 0)
                x_out()
            x_out()

            def filler(u, iv):
                x_out()
                x_chunk(u, iv + 1)

            with tc.For_i(0, NIT, 1) as iv:
                nc.sync.dma_start(
                    out=xg_it[0:JB, :].rearrange("p (t h) -> p t h", t=U),
                    in_=xg_hbm[bass.ds(iv * U, U), :, :].rearrange("t p h -> p t h"))
                for u in range(U):
                    step(u, filler=lambda u=u: filler(u, iv),
                         filler_has_mms=(6 <= u <= 13))
                nc.sync.dma_start(
                    out=hs[bass.ds(iv * U, U), :, :].rearrange("t p c -> p t c"),
                    in_=hs_it[:].rearrange("p (t c) -> p t c", t=U))

    nc.compile()
    return nc


def _prep_inputs(source, embedding, W_ih, W_hh, b, core, n_cores=NCORES):
    src_k = np.asarray(source[core * BLOC:(core + 1) * BLOC, :], dtype=np.int32)
    idx = np.ascontiguousarray(src_k.T.reshape(-1, 1))  # (t-major, b)
    idx = np.concatenate([idx, np.zeros((P, 1), np.int32)], axis=0)  # slack m-tile

    def prep_w(W, K):
        Wr = np.asarray(W, np.float32).reshape(K // P, P, 4, H)[:, :, GATE_PERM, :]
        return np.ascontiguousarray(
            Wr.transpose(1, 0, 2, 3).reshape(P, K // P, 4 * H)).astype(ml_dtypes.bfloat16)

    bias_dev = np.ascontiguousarray(
        np.asarray(b, np.float32).reshape(4, H)[GATE_PERM].reshape(4 * H))
    bias_rows = np.tile(bias_dev.reshape(4, H), (1, U))
    JB = 4 * BLOC
    scat = np.zeros((JB + 4, P), np.float32)
    for j in range(4):
        for bb in range(BLOC):
            scat[j * BLOC + bb, 32 * j + bb] = 1.0
            scat[JB + j, 32 * j + bb] = 1.0  # bias row feeds gate strip j
    return {
        "src_idx": idx,
        "emb": np.asarray(embedding, np.float32),
        "wih": prep_w(W_ih, E),
        "whh": prep_w(W_hh, H),
        "bias_rows": bias_rows.astype(ml_dtypes.bfloat16),
        "scat": scat.astype(ml_dtypes.bfloat16),
    }


def _unpack_output(hs_dev):
    KTH = H // P
    a = np.asarray(hs_dev, dtype=np.float32).reshape(S, P, KTH, BLOC)
    return np.ascontiguousarray(a.transpose(3, 0, 2, 1)).reshape(BLOC, S, H)


# Weight prep is deterministic; cache per-core input maps keyed on id of arrays.
def _get_program():
    if "nc" not in _program_cache:
        _program_cache["nc"] = build_program()
    return _program_cache["nc"]


def kernel(source, embedding, W_ih, W_hh, b):
    """Full inputs in, full output out. Shards batch over 8 NeuronCores."""
    from concourse import bass2jax

    source = np.asarray(source)
    embedding = np.asarray(embedding, np.float32)
    W_ih = np.asarray(W_ih, np.float32)
    W_hh = np.asarray(W_hh, np.float32)
    b = np.asarray(b, np.float32)

    nc = _get_program()
    in_maps = [_prep_inputs(source, embedding, W_ih, W_hh, b, core=k)
               for k in range(NCORES)]
    res = bass2jax.run_bass_via_pjrt(nc, in_maps, n_cores=NCORES)
    out = np.concatenate([_unpack_output(res[k]["hs"]) for k in range(NCORES)],
                         axis=0)
    return out.astype(np.float32)

